# revision 15
# baseline (speedup 1.0000x reference)
"""Graph multi-head attention (GNN message passing) on 8 Trainium2 NeuronCores.

Strategy v2 (dst-sharded edge parallelism, zero indirect DMAs):
  - Host: sort edges by dst, split nodes into 8 contiguous ranges with ~equal
    edge counts. Each core owns all incoming edges of its node range, so the
    per-dst segment softmax is core-local.
  - Host EXPANDS the raw per-edge operands: for every packed edge slot the
    fp16 [key||value] column of its src node, and per virtual row the fp16
    query column of its dst node. The device then projects k/q/v per edge
    with plain matmuls -- every DMA in the kernel is a large contiguous load.
  - Edges are packed into fixed-width virtual rows (node, up to D_PAD=8
    incoming edges); rows of one node stay inside one 128-row tile and are
    combined with a one-hot matmul (columns indexed by per-tile node id).
  - All projection biases are folded away exactly:
      * v-side:  v~ = value @ (Wo Wv).T  and  bo' = bo + Wo bv  (sum(alpha)=1)
      * q-side:  ones row in the q expansion + [Wq.T; bq] rhs
      * k-side:  score += q~ . bk  computed as 4 extra columns of the q matmul
        (w_h = Wq[h].T bk[h], kappa_h = bq[h].bk[h]), added to the reduced
        scores per head.
  - Segment-max subtraction is skipped (scores are O(1), exp never overflows);
    invalid slots get an additive -30000 fp16 mask before exp.
  - Output rows are stored in (tile, node-column) order; the host unshards
    with a single fancy-index per core. Degree-0 nodes are fixed up to `bo`
    on the host (device produces NaN for their empty softmax).
"""

import os
from contextlib import ExitStack

import numpy as np

N = 100000
E = 1600000
DIM = 64
H = 4
DK = DIM // H
NCORES = 8

D_PAD = 8          # edge slots per virtual row
TC = 8             # 128-row tiles per supertile
MASKV = -30000.0   # additive fp16-safe -inf
ESHIFT = -8.0      # constant exp shift: keeps exp() in fp16 range both ways


def _host_prep(src, dst):
    """Pack edges into per-core tiling metadata (no feature expansion yet)."""
    src = np.asarray(src).astype(np.int64)
    dst = np.asarray(dst).astype(np.int64)
    order = np.argsort(dst, kind="stable")
    ssrc = src[order]
    deg = np.bincount(dst, minlength=N).astype(np.int64)
    cum = np.concatenate([[0], np.cumsum(deg)])

    bounds = [0]
    for c in range(1, NCORES):
        t = round(c * E / NCORES)
        n = int(np.searchsorted(cum, t, side="left"))
        n = min(max(n, bounds[-1] + 1), N - (NCORES - c))
        bounds.append(n)
    bounds.append(N)

    packs = []
    for c in range(NCORES):
        n0, n1 = bounds[c], bounds[c + 1]
        nn = n1 - n0
        d = deg[n0:n1]
        r_n = np.maximum(1, -(-d // D_PAD)).astype(np.int64)
        tile_of = np.empty(nn, np.int64)
        colrow = np.empty(nn, np.int64)
        crow_of = np.empty(nn, np.int64)
        t_id = 0
        rows_in = 0
        nodes_in = 0
        for i in range(nn):
            r = r_n[i]
            if rows_in + r > 128:
                t_id += 1
                rows_in = 0
                nodes_in = 0
            tile_of[i] = t_id
            colrow[i] = rows_in
            crow_of[i] = nodes_in
            rows_in += r
            nodes_in += 1
        packs.append(dict(n0=n0, n1=n1, nn=nn, d=d, r_n=r_n, tile_of=tile_of,
                          colrow=colrow, crow_of=crow_of, nt=t_id + 1))

    NT = -(-max(p["nt"] for p in packs) // TC) * TC
    return packs, ssrc, cum, NT


def _expand_core(p, ssrc, cum, NT, keyT16, valT16, qT16):
    """Build the per-core expanded fp16 operand arrays."""
    n0 = p["n0"]
    nn = p["nn"]
    d, r_n = p["d"], p["r_n"]
    rows_total = NT * 128

    row_node = np.repeat(np.arange(nn), r_n)
    starts = np.concatenate([[0], np.cumsum(r_n)])[:-1]
    row_k = np.arange(len(row_node)) - np.repeat(starts, r_n)
    row_slot = (np.repeat(p["tile_of"], r_n) * 128
                + np.repeat(p["colrow"], r_n) + row_k)
    row_deg = np.clip(np.repeat(d, r_n) - row_k * D_PAD, 0, D_PAD)
    row_e0 = cum[n0 + row_node] + row_k * D_PAD
    j = np.arange(D_PAD)[None, :]
    valid = j < row_deg[:, None]
    eidx = np.minimum(row_e0[:, None] + j, E - 1)
    srcv = ssrc[eidx]

    # kvx[0:64, col]=key.T[src], [64:128]=value.T[src]; col=T*1024+s*128+p
    kvx = np.zeros((128, NT * 1024), np.float16)
    T_of = row_slot // 128
    p_of = row_slot % 128
    cols = T_of[:, None] * 1024 + j * 128 + p_of[:, None]
    cv = cols[valid]
    sv = srcv[valid]
    kvx[0:64, cv] = keyT16[:, sv]
    kvx[64:128, cv] = valT16[:, sv]

    # qx [65, NT*128], ones row for bias folding
    qx = np.zeros((65, NT * 128), np.float16)
    qx[64, :] = 1.0
    qx[0:64, row_slot] = qT16[:, n0 + row_node]

    # additive mask [128, NT*32], col = T*32 + s*4 + h
    mrow = np.full((rows_total, D_PAD), MASKV, np.float16)
    mrow[row_slot] = np.where(valid, np.float16(0.0), np.float16(MASKV))
    m4 = np.repeat(mrow.reshape(NT, 128, D_PAD)[:, :, :, None], H, axis=3)
    msk = np.ascontiguousarray(
        m4.transpose(1, 0, 2, 3).reshape(128, NT * D_PAD * H))

    # per-row node-column id [128, NT]
    crow_slot = np.zeros(rows_total, np.int32)
    crow_slot[row_slot] = np.repeat(p["crow_of"], r_n).astype(np.int32)
    crw = np.ascontiguousarray(crow_slot.reshape(NT, 128).T)

    return dict(kvx=kvx, qx=qx, msk=msk, crw=crw)


def _build_program(NT):
    import concourse.bass as bass
    import concourse.tile as tile
    from concourse import bacc, mybir

    f32 = mybir.dt.float32
    f16 = mybir.dt.float16
    i32 = mybir.dt.int32
    AO = mybir.AluOpType

    nc = bacc.Bacc("TRN2", target_bir_lowering=False, debug=False,
                   num_devices=NCORES)

    kvx = nc.dram_tensor("kvx", [128, NT * 1024], f16, kind="ExternalInput").ap()
    qx = nc.dram_tensor("qx", [65, NT * 128], f16, kind="ExternalInput").ap()
    mskd = nc.dram_tensor("msk", [128, NT * D_PAD * H], f16, kind="ExternalInput").ap()
    crwd = nc.dram_tensor("crw", [128, NT], i32, kind="ExternalInput").ap()
    wkvd = nc.dram_tensor("wkv", [128, 128], f16, kind="ExternalInput").ap()
    wqd = nc.dram_tensor("wq", [65, 68], f16, kind="ExternalInput").ap()
    wod = nc.dram_tensor("wo", [DIM, DIM], f16, kind="ExternalInput").ap()
    comb = nc.dram_tensor("comb", [NT * 128, DIM], f16, kind="ExternalOutput").ap()

    ST = NT // TC

    def apx(t, dims, extra_off=0):
        a = t[:]
        return bass.AP(a.tensor, a.offset + extra_off, [list(a.ap[0])] + dims)

    with tile.TileContext(nc) as tc, ExitStack() as ctx, \
            nc.allow_low_precision("fp16 edge softmax within 2e-2 tolerance"):
        consts = ctx.enter_context(tc.tile_pool(name="consts", bufs=1))
        ld = ctx.enter_context(tc.tile_pool(name="ld", bufs=3))
        work = ctx.enter_context(tc.tile_pool(name="work", bufs=3))
        bpool = ctx.enter_context(tc.tile_pool(name="bpool", bufs=3))
        bwork = ctx.enter_context(tc.tile_pool(name="bwork", bufs=4))
        pstp = ctx.enter_context(tc.tile_pool(name="pstp", bufs=2, space="PSUM"))
        qpsp = ctx.enter_context(tc.tile_pool(name="qpsp", bufs=2, space="PSUM"))
        cpsp = ctx.enter_context(tc.tile_pool(name="cpsp", bufs=2, space="PSUM"))

        from concourse.masks import make_identity

        wkv_sb = consts.tile([128, 128], f16)
        nc.sync.dma_start(wkv_sb[:], wkvd[:, :])
        wq_sb = consts.tile([65, 68], f16)
        nc.sync.dma_start(wq_sb[:], wqd[:, :])
        wo_sb = consts.tile([DIM, DIM], f16)
        nc.sync.dma_start(wo_sb[:], wod[:, :])
        iota_i = consts.tile([128, 128], i32)
        nc.gpsimd.iota(iota_i[:], pattern=[[1, 128]], base=0, channel_multiplier=0)
        iota_f = consts.tile([128, 128], f16)
        nc.vector.tensor_copy(iota_f[:], iota_i[:])
        ident = consts.tile([128, 128], f16)
        make_identity(nc, ident[:])
        esh = consts.tile([128, 1], f32)
        nc.vector.memset(esh[:], ESHIFT)

        # state carried from supertile st-1 for deferred combine phase
        prev = None

        def emit_A(st):
            kvld = ld.tile([128, TC * 1024], f16, tag="kvld")
            nc.sync.dma_start(kvld[:], kvx[:, st * TC * 1024:(st + 1) * TC * 1024])
            qld = ld.tile([65, TC * 128], f16, tag="qld")
            nc.scalar.dma_start(qld[:], qx[:, st * TC * 128:(st + 1) * TC * 128])
            mld = ld.tile([128, TC * D_PAD * H], f16, tag="mld")
            nc.scalar.dma_start(
                mld[:], mskd[:, st * TC * D_PAD * H:(st + 1) * TC * D_PAD * H])
            cld = ld.tile([128, TC], i32, tag="cld")
            nc.scalar.dma_start(cld[:], crwd[:, st * TC:(st + 1) * TC])
            crwf = ld.tile([128, TC], f16, tag="crwf")
            nc.vector.tensor_copy(crwf[:], cld[:])

            q16 = work.tile([128, TC, 68], f16, tag="q16")
            prodt = work.tile([128, TC, D_PAD, DIM], f16, tag="prod")
            vt16 = bpool.tile([128, TC, D_PAD, DIM], f16, tag="vt16")
            sco = work.tile([128, TC, D_PAD, H], f16, tag="sco")
            adex = bpool.tile([128, TC, D_PAD, DIM + H], f16, tag="adex")

            for t in range(TC):
                # q~ projection: [128 rows, 68] (64 feats + 4 bk-fold cols)
                qp = qpsp.tile([128, 68], f32, space="PSUM", tag="qps")
                nc.tensor.matmul(out=qp[:], lhsT=qld[:, t * 128:(t + 1) * 128],
                                 rhs=wq_sb[:], start=True, stop=True)
                nc.scalar.copy(q16[:, t, :], qp[:])

                # per-edge [k^ || v] projection: 8 slots -> PSUM [128,8,128]
                pst = pstp.tile([128, D_PAD, 128], f32, space="PSUM", tag="pst")
                for sl in range(D_PAD):
                    nc.tensor.matmul(
                        out=pst[:, sl, :],
                        lhsT=kvld[:, (t * D_PAD + sl) * 128:
                                  (t * D_PAD + sl + 1) * 128],
                        rhs=wkv_sb[:], start=True, stop=True)
                # prod = k^ * q~ (slot-bcast); drain v half to fp16
                nc.vector.tensor_tensor(
                    out=prodt[:, t, :, :],
                    in0=apx(pst, [[128, D_PAD], [1, DIM]]),
                    in1=apx(q16, [[0, D_PAD], [1, DIM]], extra_off=t * 68),
                    op=AO.mult)
                nc.scalar.copy(vt16[:, t, :, :],
                               apx(pst, [[128, D_PAD], [1, DIM]], extra_off=DIM))

            # supertile-batched score pipeline
            nc.vector.tensor_reduce(
                out=sco[:],
                in_=apx(prodt, [[D_PAD * DIM, TC], [DIM, D_PAD], [DK, H], [1, DK]]),
                axis=mybir.AxisListType.X, op=AO.add)
            nc.gpsimd.tensor_tensor(
                out=sco[:], in0=sco[:],
                in1=apx(mld, [[D_PAD * H, TC], [H, D_PAD], [1, H]]),
                op=AO.add)
            nc.gpsimd.tensor_tensor(
                out=sco[:], in0=sco[:],
                in1=apx(q16, [[68, TC], [0, D_PAD], [1, H]], extra_off=DIM),
                op=AO.add)
            nc.scalar.activation(
                out=apx(adex, [[(DIM + H) * D_PAD, TC], [DIM + H, D_PAD], [1, H]],
                        extra_off=DIM),
                in_=sco[:],
                func=mybir.ActivationFunctionType.Exp,
                scale=1.0 / np.sqrt(DK), bias=esh[:])
            # wv = v * exp, reading the just-written exp values (broadcast 16x)
            nc.gpsimd.tensor_tensor(
                out=apx(adex, [[(DIM + H) * D_PAD, TC], [DIM + H, D_PAD],
                               [DK, H], [1, DK]]),
                in0=apx(vt16, [[D_PAD * DIM, TC], [DIM, D_PAD], [DK, H], [1, DK]]),
                in1=apx(adex, [[(DIM + H) * D_PAD, TC], [DIM + H, D_PAD],
                               [1, H], [0, DK]], extra_off=DIM),
                op=AO.mult)
            oh = bpool.tile([128, TC, 128], f16, tag="oh")
            nc.vector.tensor_tensor(
                out=oh[:], in0=apx(iota_f, [[0, TC], [1, 128]]),
                in1=apx(crwf, [[1, TC], [0, 128]]),
                op=AO.is_equal)
            return dict(st=st, adex=adex, oh=oh)

        def emit_B(state):
            st = state["st"]
            adex = state["adex"]
            oh = state["oh"]
            for t in range(TC):
                # one PSUM bank per tile for the whole B chain:
                # f32 [0:136] slot-pair combine, f16 [272:400] transpose out,
                # f32 [200:264] Wo matmul out
                mega = cpsp.tile([128, 512], f32, space="PSUM", tag="bmega")
                cp = apx(mega, [[DIM + H, 2], [1, DIM + H]])
                for s4 in range(4):
                    nc.tensor.matmul(
                        out=cp, lhsT=oh[:, t, :],
                        rhs=adex[:, t, 2 * s4:2 * s4 + 2, :],
                        start=(s4 == 0), stop=(s4 == 3))
                cpf = bwork.tile([128, DIM + H], f32, tag="cpf")
                nc.vector.tensor_reduce(
                    out=cpf[:],
                    in_=apx(mega, [[1, DIM + H], [DIM + H, 2]]),
                    axis=mybir.AxisListType.X, op=AO.add)
                rd = bwork.tile([128, H], f32, tag="rd")
                nc.vector.reciprocal(rd[:], cpf[:, DIM:DIM + H])
                nrm = bwork.tile([128, DIM], f16, tag="nrm")
                nc.gpsimd.tensor_tensor(
                    out=apx(nrm, [[DK, H], [1, DK]]),
                    in0=apx(cpf, [[DK, H], [1, DK]]),
                    in1=apx(rd, [[1, H], [0, DK]]),
                    op=AO.mult)
                # transpose + output projection (Wo mixes heads, so it must
                # run after the per-head normalization)
                meg16 = mega.bitcast(f16)
                tps = meg16[0:DIM, 272:400]
                nc.tensor.transpose(out=tps, in_=nrm[:], identity=ident[:])
                nrmT = bwork.tile([DIM, 128], f16, tag="nrmT")
                nc.scalar.copy(nrmT[:], tps)
                ops_ = mega[:, 200:264]
                nc.tensor.matmul(out=ops_, lhsT=nrmT[:], rhs=wo_sb[:],
                                 start=True, stop=True)
                osb = bwork.tile([128, DIM], f16, tag="osb")
                nc.scalar.copy(osb[:], ops_)
                T = st * TC + t
                nc.sync.dma_start(comb[T * 128:(T + 1) * 128, :], osb[:])

        for st in range(ST):
            state = emit_A(st)
            if prev is not None:
                emit_B(prev)
            prev = state
        emit_B(prev)

    nc.compile()
    return nc


def kernel(**inputs):
    from concourse.bass_utils import run_bass_kernel_spmd

    query = np.asarray(inputs["query"], np.float32)
    key = np.asarray(inputs["key"], np.float32)
    value = np.asarray(inputs["value"], np.float32)
    src = np.asarray(inputs["src"])
    dst = np.asarray(inputs["dst"])
    Wq = np.asarray(inputs["Wq"], np.float32)
    bq = np.asarray(inputs["bq"], np.float32)
    Wk = np.asarray(inputs["Wk"], np.float32)
    bk = np.asarray(inputs["bk"], np.float32)
    Wv = np.asarray(inputs["Wv"], np.float32)
    bv = np.asarray(inputs["bv"], np.float32)
    Wo = np.asarray(inputs["Wo"], np.float32)
    bo = np.asarray(inputs["bo"], np.float32)

    packs, ssrc, cum, NT = _host_prep(src, dst)
    nc = _build_program(NT)

    keyT16 = np.ascontiguousarray(key.T).astype(np.float16)
    valT16 = np.ascontiguousarray(value.T).astype(np.float16)
    qT16 = np.ascontiguousarray(query.T).astype(np.float16)

    # weight packing with bias folding (v stays in head space; Wo is applied
    # on-device after the per-head normalization)
    wkv = np.zeros((128, 128), np.float16)
    wkv[0:64, 0:64] = Wk.T
    wkv[64:128, 64:128] = Wv.T
    wq = np.zeros((65, 68), np.float16)
    wq[0:64, 0:64] = Wq.T
    wq[64, 0:64] = bq
    for h in range(H):
        sl = slice(h * DK, (h + 1) * DK)
        wq[0:64, 64 + h] = Wq[sl, :].T @ bk[sl]
        wq[64, 64 + h] = bq[sl] @ bk[sl]
    bo_eff = (bo + Wo @ bv).astype(np.float32)

    in_maps = []
    for p in packs:
        ex = _expand_core(p, ssrc, cum, NT, keyT16, valT16, qT16)
        in_maps.append(dict(kvx=ex["kvx"], qx=ex["qx"], msk=ex["msk"],
                            crw=ex["crw"], wkv=wkv, wq=wq,
                            wo=Wo.T.astype(np.float16)))

    trace = bool(int(os.environ.get("KERNEL_TRACE", "0")))
    res = run_bass_kernel_spmd(
        nc, in_maps, core_ids=list(range(NCORES)), trace=trace,
        tmpdir=os.environ.get("KERNEL_TRACE_DIR") or None,
    )
    kernel.last_results = res

    out = np.empty((N, DIM), np.float32)
    for p, r in zip(packs, res.results):
        rows = p["tile_of"] * 128 + p["crow_of"]
        out[p["n0"]:p["n1"]] = r["comb"][rows].astype(np.float32) + bo_eff
        z = p["d"] == 0
        if z.any():
            out[p["n0"]:p["n1"]][z] = bo
    return out


# revision 17
# speedup vs baseline: 1.3096x; 1.3096x over previous
"""Graph multi-head attention (GNN message passing) on 8 Trainium2 NeuronCores.

Strategy v2 (dst-sharded edge parallelism, zero indirect DMAs):
  - Host: sort edges by dst, split nodes into 8 contiguous ranges with ~equal
    edge counts. Each core owns all incoming edges of its node range, so the
    per-dst segment softmax is core-local.
  - Host EXPANDS the raw per-edge operands: for every packed edge slot the
    fp16 [key||value] column of its src node, and per virtual row the fp16
    query column of its dst node. The device then projects k/q/v per edge
    with plain matmuls -- every DMA in the kernel is a large contiguous load.
  - Edges are packed into fixed-width virtual rows (node, up to D_PAD=8
    incoming edges); rows of one node stay inside one 128-row tile and are
    combined with a one-hot matmul (columns indexed by per-tile node id).
  - All projection biases are folded away exactly:
      * v-side:  v~ = value @ (Wo Wv).T  and  bo' = bo + Wo bv  (sum(alpha)=1)
      * q-side:  ones row in the q expansion + [Wq.T; bq] rhs
      * k-side:  score += q~ . bk  computed as 4 extra columns of the q matmul
        (w_h = Wq[h].T bk[h], kappa_h = bq[h].bk[h]), added to the reduced
        scores per head.
  - Segment-max subtraction is skipped (scores are O(1), exp never overflows);
    invalid slots get an additive -30000 fp16 mask before exp.
  - Output rows are stored in (tile, node-column) order; the host unshards
    with a single fancy-index per core. Degree-0 nodes are fixed up to `bo`
    on the host (device produces NaN for their empty softmax).
"""

import os
from contextlib import ExitStack

import numpy as np

N = 100000
E = 1600000
DIM = 64
H = 4
DK = DIM // H
NCORES = 8

D_PAD = 8          # edge slots per virtual row
TC = 8             # 128-row tiles per supertile
MASKV = -30000.0   # additive fp16-safe -inf
ESHIFT = -8.0      # constant exp shift: keeps exp() in fp16 range both ways


def _host_prep(src, dst):
    """Pack edges into per-core tiling metadata (no feature expansion yet)."""
    src = np.asarray(src).astype(np.int64)
    dst = np.asarray(dst).astype(np.int64)
    order = np.argsort(dst, kind="stable")
    ssrc = src[order]
    deg = np.bincount(dst, minlength=N).astype(np.int64)
    cum = np.concatenate([[0], np.cumsum(deg)])

    bounds = [0]
    for c in range(1, NCORES):
        t = round(c * E / NCORES)
        n = int(np.searchsorted(cum, t, side="left"))
        n = min(max(n, bounds[-1] + 1), N - (NCORES - c))
        bounds.append(n)
    bounds.append(N)

    packs = []
    for c in range(NCORES):
        n0, n1 = bounds[c], bounds[c + 1]
        nn = n1 - n0
        d = deg[n0:n1]
        r_n = np.maximum(1, -(-d // D_PAD)).astype(np.int64)
        tile_of = np.empty(nn, np.int64)
        colrow = np.empty(nn, np.int64)
        crow_of = np.empty(nn, np.int64)
        t_id = 0
        rows_in = 0
        nodes_in = 0
        for i in range(nn):
            r = r_n[i]
            if rows_in + r > 128:
                t_id += 1
                rows_in = 0
                nodes_in = 0
            tile_of[i] = t_id
            colrow[i] = rows_in
            crow_of[i] = nodes_in
            rows_in += r
            nodes_in += 1
        packs.append(dict(n0=n0, n1=n1, nn=nn, d=d, r_n=r_n, tile_of=tile_of,
                          colrow=colrow, crow_of=crow_of, nt=t_id + 1))

    NT = -(-max(p["nt"] for p in packs) // TC) * TC
    return packs, ssrc, cum, NT


def _expand_core(p, ssrc, cum, NT, keyT16, valT16, qT16):
    """Build the per-core expanded fp16 operand arrays."""
    n0 = p["n0"]
    nn = p["nn"]
    d, r_n = p["d"], p["r_n"]
    rows_total = NT * 128

    row_node = np.repeat(np.arange(nn), r_n)
    starts = np.concatenate([[0], np.cumsum(r_n)])[:-1]
    row_k = np.arange(len(row_node)) - np.repeat(starts, r_n)
    row_slot = (np.repeat(p["tile_of"], r_n) * 128
                + np.repeat(p["colrow"], r_n) + row_k)
    row_deg = np.clip(np.repeat(d, r_n) - row_k * D_PAD, 0, D_PAD)
    row_e0 = cum[n0 + row_node] + row_k * D_PAD
    j = np.arange(D_PAD)[None, :]
    valid = j < row_deg[:, None]
    eidx = np.minimum(row_e0[:, None] + j, E - 1)
    srcv = ssrc[eidx]

    # kvx[0:64, col]=key.T[src], [64:128]=value.T[src]; col=T*1024+s*128+p
    kvx = np.zeros((128, NT * 1024), np.float16)
    T_of = row_slot // 128
    p_of = row_slot % 128
    cols = T_of[:, None] * 1024 + j * 128 + p_of[:, None]
    cv = cols[valid]
    sv = srcv[valid]
    kvx[0:64, cv] = keyT16[:, sv]
    kvx[64:128, cv] = valT16[:, sv]

    # qx [65, NT*128], ones row for bias folding
    qx = np.zeros((65, NT * 128), np.float16)
    qx[64, :] = 1.0
    qx[0:64, row_slot] = qT16[:, n0 + row_node]

    # additive mask [128, NT*32], col = T*32 + s*4 + h
    mrow = np.full((rows_total, D_PAD), MASKV, np.float16)
    mrow[row_slot] = np.where(valid, np.float16(0.0), np.float16(MASKV))
    m4 = np.repeat(mrow.reshape(NT, 128, D_PAD)[:, :, :, None], H, axis=3)
    msk = np.ascontiguousarray(
        m4.transpose(1, 0, 2, 3).reshape(128, NT * D_PAD * H))

    # per-row node-column one-hot [128, NT*128] (host-built, DMA'd in)
    crow_slot = np.zeros(rows_total, np.int32)
    crow_slot[row_slot] = np.repeat(p["crow_of"], r_n).astype(np.int32)
    crw = crow_slot.reshape(NT, 128).T  # [p, T]
    ohx = np.ascontiguousarray(
        (crw[:, :, None] == np.arange(128)[None, None, :])
        .astype(np.float16).reshape(128, NT * 128))

    return dict(kvx=kvx, qx=qx, msk=msk, ohx=ohx)


def _build_program(NT):
    import concourse.bass as bass
    import concourse.tile as tile
    from concourse import bacc, mybir

    f32 = mybir.dt.float32
    f16 = mybir.dt.float16
    i32 = mybir.dt.int32
    AO = mybir.AluOpType

    nc = bacc.Bacc("TRN2", target_bir_lowering=False, debug=False,
                   num_devices=NCORES)

    kvx = nc.dram_tensor("kvx", [128, NT * 1024], f16, kind="ExternalInput").ap()
    qx = nc.dram_tensor("qx", [65, NT * 128], f16, kind="ExternalInput").ap()
    mskd = nc.dram_tensor("msk", [128, NT * D_PAD * H], f16, kind="ExternalInput").ap()
    ohxd = nc.dram_tensor("ohx", [128, NT * 128], f16, kind="ExternalInput").ap()
    wkvd = nc.dram_tensor("wkv", [128, 128], f16, kind="ExternalInput").ap()
    wqd = nc.dram_tensor("wq", [65, 68], f16, kind="ExternalInput").ap()
    wod = nc.dram_tensor("wo", [DIM, DIM], f16, kind="ExternalInput").ap()
    bod = nc.dram_tensor("bo", [128, DIM], f32, kind="ExternalInput").ap()
    comb = nc.dram_tensor("comb", [NT * 128, DIM], f32, kind="ExternalOutput").ap()

    ST = NT // TC

    def apx(t, dims, extra_off=0):
        a = t[:]
        return bass.AP(a.tensor, a.offset + extra_off, [list(a.ap[0])] + dims)

    with tile.TileContext(nc) as tc, ExitStack() as ctx, \
            nc.allow_low_precision("fp16 edge softmax within 2e-2 tolerance"):
        consts = ctx.enter_context(tc.tile_pool(name="consts", bufs=1))
        ld = ctx.enter_context(tc.tile_pool(name="ld", bufs=3))
        work = ctx.enter_context(tc.tile_pool(name="work", bufs=3))
        adexp = ctx.enter_context(tc.tile_pool(name="adexp", bufs=2 * TC + 2))
        bwork = ctx.enter_context(tc.tile_pool(name="bwork", bufs=4))
        pstp = ctx.enter_context(tc.tile_pool(name="pstp", bufs=2, space="PSUM"))
        qpsp = ctx.enter_context(tc.tile_pool(name="qpsp", bufs=2, space="PSUM"))
        cpsp = ctx.enter_context(tc.tile_pool(name="cpsp", bufs=2, space="PSUM"))

        from concourse.masks import make_identity

        wkv_sb = consts.tile([128, 128], f16)
        nc.sync.dma_start(wkv_sb[:], wkvd[:, :])
        wq_sb = consts.tile([65, 68], f16)
        nc.sync.dma_start(wq_sb[:], wqd[:, :])
        wo_sb = consts.tile([DIM, DIM], f16)
        nc.sync.dma_start(wo_sb[:], wod[:, :])
        bo_sb = consts.tile([128, DIM], f32)
        nc.sync.dma_start(bo_sb[:], bod[:, :])
        ident = consts.tile([128, 128], f16)
        make_identity(nc, ident[:])
        esh = consts.tile([128, 1], f32)
        nc.vector.memset(esh[:], ESHIFT)

        # state carried from supertile st-1 for deferred combine phase
        prev = None

        def emit_A(st):
            kvld = ld.tile([128, TC * 1024], f16, tag="kvld")
            nc.sync.dma_start(kvld[:], kvx[:, st * TC * 1024:(st + 1) * TC * 1024])
            qld = ld.tile([65, TC * 128], f16, tag="qld")
            nc.scalar.dma_start(qld[:], qx[:, st * TC * 128:(st + 1) * TC * 128])
            mld = ld.tile([128, TC * D_PAD * H], f16, tag="mld")
            nc.scalar.dma_start(
                mld[:], mskd[:, st * TC * D_PAD * H:(st + 1) * TC * D_PAD * H])
            ohld = ld.tile([128, TC * 128], f16, tag="ohld")
            nc.scalar.dma_start(ohld[:], ohxd[:, st * TC * 128:(st + 1) * TC * 128])

            adex_l = []
            for t in range(TC):
                # q~ projection: [128 rows, 68] (64 feats + 4 bk-fold cols)
                qp = qpsp.tile([128, 68], f32, space="PSUM", tag="qps")
                nc.tensor.matmul(out=qp[:], lhsT=qld[:, t * 128:(t + 1) * 128],
                                 rhs=wq_sb[:], start=True, stop=True)
                q16 = work.tile([128, 68], f16, tag="q16")
                nc.scalar.copy(q16[:], qp[:])

                # per-edge [k^ || v] projection: 8 slots -> PSUM [128,8,128]
                pst = pstp.tile([128, D_PAD, 128], f32, space="PSUM", tag="pst")
                for sl in range(D_PAD):
                    nc.tensor.matmul(
                        out=pst[:, sl, :],
                        lhsT=kvld[:, (t * D_PAD + sl) * 128:
                                  (t * D_PAD + sl + 1) * 128],
                        rhs=wkv_sb[:], start=True, stop=True)

                # scores: prod = k^ * q~ (slot-bcast), reduce per head
                prod = work.tile([128, D_PAD, DIM], f16, tag="prod")
                nc.vector.tensor_tensor(
                    out=prod[:],
                    in0=apx(pst, [[128, D_PAD], [1, DIM]]),
                    in1=apx(q16, [[0, D_PAD], [1, DIM]]),
                    op=AO.mult)
                sco = work.tile([128, D_PAD, H], f16, tag="sco")
                nc.vector.tensor_reduce(
                    out=sco[:],
                    in_=apx(prod, [[DIM, D_PAD], [DK, H], [1, DK]]),
                    axis=mybir.AxisListType.X, op=AO.add)
                # + mask, + per-head q.bk correction
                nc.vector.tensor_tensor(
                    out=sco[:], in0=sco[:],
                    in1=apx(mld, [[H, D_PAD], [1, H]], extra_off=t * D_PAD * H),
                    op=AO.add)
                nc.gpsimd.tensor_tensor(
                    out=sco[:], in0=sco[:],
                    in1=apx(q16, [[0, D_PAD], [1, H]], extra_off=DIM),
                    op=AO.add)

                # exp (scale 1/sqrt(dk), constant shift); adex = [exp*v || exp]
                adex = adexp.tile([128, D_PAD, DIM + H], f16, tag="adex")
                exe = work.tile([128, D_PAD, DIM], f16, tag="exe")
                nc.scalar.activation(
                    out=exe[:],
                    in_=apx(sco, [[H, D_PAD], [1, H], [0, DK]]),
                    func=mybir.ActivationFunctionType.Exp,
                    scale=1.0 / np.sqrt(DK), bias=esh[:])
                nc.scalar.activation(
                    out=apx(adex, [[DIM + H, D_PAD], [1, H]], extra_off=DIM),
                    in_=sco[:],
                    func=mybir.ActivationFunctionType.Exp,
                    scale=1.0 / np.sqrt(DK), bias=esh[:])
                vt16 = work.tile([128, D_PAD, DIM], f16, tag="vt16")
                nc.scalar.copy(vt16[:], apx(pst, [[128, D_PAD], [1, DIM]],
                                            extra_off=DIM))
                nc.gpsimd.tensor_tensor(
                    out=apx(adex, [[DIM + H, D_PAD], [1, DIM]]),
                    in0=vt16[:], in1=exe[:], op=AO.mult)
                adex_l.append(adex)
            return dict(st=st, adex=adex_l, ohld=ohld)

        def emit_B(state):
            st = state["st"]
            ohld = state["ohld"]
            for t in range(TC):
                adex = state["adex"][t]
                oh = ohld[:, t * 128:(t + 1) * 128]
                # one PSUM bank shared by the whole B chain:
                # f32 elems [0:136] = slot-pair combine, f16 elems [272:400]
                # = transpose out, f32 elems [200:264] = Wo matmul out
                mega = cpsp.tile([128, 512], f32, space="PSUM", tag="bmega")
                cp = apx(mega, [[DIM + H, 2], [1, DIM + H]])
                meg16 = mega.bitcast(f16)
                # combine rows -> node columns: 4 chained matmuls over slot
                # pairs, pairwise sums land in [128, 2, 68]
                for s4 in range(4):
                    nc.tensor.matmul(
                        out=cp, lhsT=oh,
                        rhs=adex[:, 2 * s4:2 * s4 + 2, :],
                        start=(s4 == 0), stop=(s4 == 3))
                cpf = bwork.tile([128, DIM + H], f32, tag="cpf")
                nc.vector.tensor_reduce(
                    out=cpf[:],
                    in_=apx(mega, [[1, DIM + H], [DIM + H, 2]]),
                    axis=mybir.AxisListType.X, op=AO.add)
                rd = bwork.tile([128, H], f32, tag="rd")
                nc.vector.reciprocal(rd[:], cpf[:, DIM:DIM + H])
                nrm = bwork.tile([128, DIM], f16, tag="nrm")
                nc.gpsimd.tensor_tensor(
                    out=nrm[:], in0=cpf[:, 0:DIM],
                    in1=apx(rd, [[1, H], [0, DK]]),
                    op=AO.mult)
                # transpose + output projection (Wo mixes heads, so it must
                # run after the per-head normalization)
                tps = meg16[0:DIM, 272:400]
                nc.tensor.transpose(out=tps, in_=nrm[:], identity=ident[:])
                nrmT = bwork.tile([DIM, 128], f16, tag="nrmT")
                nc.scalar.copy(nrmT[:], tps)
                ops_ = mega[:, 200:264]
                nc.tensor.matmul(out=ops_, lhsT=nrmT[:], rhs=wo_sb[:],
                                 start=True, stop=True)
                osb = bwork.tile([128, DIM], f32, tag="osb")
                nc.vector.tensor_tensor(
                    out=osb[:], in0=ops_, in1=bo_sb[:], op=AO.add)
                T = st * TC + t
                nc.sync.dma_start(comb[T * 128:(T + 1) * 128, :], osb[:])

        for st in range(ST):
            state = emit_A(st)
            if prev is not None:
                emit_B(prev)
            prev = state
        emit_B(prev)

    nc.compile()
    return nc


def kernel(**inputs):
    from concourse.bass_utils import run_bass_kernel_spmd

    query = np.asarray(inputs["query"], np.float32)
    key = np.asarray(inputs["key"], np.float32)
    value = np.asarray(inputs["value"], np.float32)
    src = np.asarray(inputs["src"])
    dst = np.asarray(inputs["dst"])
    Wq = np.asarray(inputs["Wq"], np.float32)
    bq = np.asarray(inputs["bq"], np.float32)
    Wk = np.asarray(inputs["Wk"], np.float32)
    bk = np.asarray(inputs["bk"], np.float32)
    Wv = np.asarray(inputs["Wv"], np.float32)
    bv = np.asarray(inputs["bv"], np.float32)
    Wo = np.asarray(inputs["Wo"], np.float32)
    bo = np.asarray(inputs["bo"], np.float32)

    packs, ssrc, cum, NT = _host_prep(src, dst)
    nc = _build_program(NT)

    keyT16 = np.ascontiguousarray(key.T).astype(np.float16)
    valT16 = np.ascontiguousarray(value.T).astype(np.float16)
    qT16 = np.ascontiguousarray(query.T).astype(np.float16)

    # weight packing with bias folding (v stays in head space; Wo is applied
    # on-device after the per-head normalization)
    wkv = np.zeros((128, 128), np.float16)
    wkv[0:64, 0:64] = Wk.T
    wkv[64:128, 64:128] = Wv.T
    wq = np.zeros((65, 68), np.float16)
    wq[0:64, 0:64] = Wq.T
    wq[64, 0:64] = bq
    for h in range(H):
        sl = slice(h * DK, (h + 1) * DK)
        wq[0:64, 64 + h] = Wq[sl, :].T @ bk[sl]
        wq[64, 64 + h] = bq[sl] @ bk[sl]
    bo_eff = (bo + Wo @ bv).astype(np.float32)
    bo_b = np.broadcast_to(bo_eff, (128, DIM)).astype(np.float32).copy()

    in_maps = []
    for p in packs:
        ex = _expand_core(p, ssrc, cum, NT, keyT16, valT16, qT16)
        in_maps.append(dict(kvx=ex["kvx"], qx=ex["qx"], msk=ex["msk"],
                            ohx=ex["ohx"], wkv=wkv, wq=wq,
                            wo=Wo.T.astype(np.float16), bo=bo_b))

    trace = bool(int(os.environ.get("KERNEL_TRACE", "0")))
    res = run_bass_kernel_spmd(
        nc, in_maps, core_ids=list(range(NCORES)), trace=trace,
        tmpdir=os.environ.get("KERNEL_TRACE_DIR") or None,
    )
    kernel.last_results = res

    out = np.empty((N, DIM), np.float32)
    for p, r in zip(packs, res.results):
        rows = p["tile_of"] * 128 + p["crow_of"]
        out[p["n0"]:p["n1"]] = r["comb"][rows]
        z = p["d"] == 0
        if z.any():
            out[p["n0"]:p["n1"]][z] = bo
    return out


# revision 18
# speedup vs baseline: 1.3361x; 1.0203x over previous
"""Graph multi-head attention (GNN message passing) on 8 Trainium2 NeuronCores.

Strategy v2 (dst-sharded edge parallelism, zero indirect DMAs):
  - Host: sort edges by dst, split nodes into 8 contiguous ranges with ~equal
    edge counts. Each core owns all incoming edges of its node range, so the
    per-dst segment softmax is core-local.
  - Host EXPANDS the raw per-edge operands: for every packed edge slot the
    fp16 [key||value] column of its src node, and per virtual row the fp16
    query column of its dst node. The device then projects k/q/v per edge
    with plain matmuls -- every DMA in the kernel is a large contiguous load.
  - Edges are packed into fixed-width virtual rows (node, up to D_PAD=8
    incoming edges); rows of one node stay inside one 128-row tile and are
    combined with a one-hot matmul (columns indexed by per-tile node id).
  - All projection biases are folded away exactly:
      * v-side:  v~ = value @ (Wo Wv).T  and  bo' = bo + Wo bv  (sum(alpha)=1)
      * q-side:  ones row in the q expansion + [Wq.T; bq] rhs
      * k-side:  score += q~ . bk  computed as 4 extra columns of the q matmul
        (w_h = Wq[h].T bk[h], kappa_h = bq[h].bk[h]), added to the reduced
        scores per head.
  - Segment-max subtraction is skipped (scores are O(1), exp never overflows);
    invalid slots get an additive -30000 fp16 mask before exp.
  - Output rows are stored in (tile, node-column) order; the host unshards
    with a single fancy-index per core. Degree-0 nodes are fixed up to `bo`
    on the host (device produces NaN for their empty softmax).
"""

import os
from contextlib import ExitStack

import numpy as np

N = 100000
E = 1600000
DIM = 64
H = 4
DK = DIM // H
NCORES = 8

D_PAD = 8          # edge slots per virtual row
TC = 8             # 128-row tiles per supertile
MASKV = -30000.0   # additive fp16-safe -inf
ESHIFT = -8.0      # constant exp shift: keeps exp() in fp16 range both ways


def _host_prep(src, dst):
    """Pack edges into per-core tiling metadata (no feature expansion yet)."""
    src = np.asarray(src).astype(np.int64)
    dst = np.asarray(dst).astype(np.int64)
    order = np.argsort(dst, kind="stable")
    ssrc = src[order]
    deg = np.bincount(dst, minlength=N).astype(np.int64)
    cum = np.concatenate([[0], np.cumsum(deg)])

    bounds = [0]
    for c in range(1, NCORES):
        t = round(c * E / NCORES)
        n = int(np.searchsorted(cum, t, side="left"))
        n = min(max(n, bounds[-1] + 1), N - (NCORES - c))
        bounds.append(n)
    bounds.append(N)

    packs = []
    for c in range(NCORES):
        n0, n1 = bounds[c], bounds[c + 1]
        nn = n1 - n0
        d = deg[n0:n1]
        r_n = np.maximum(1, -(-d // D_PAD)).astype(np.int64)
        tile_of = np.empty(nn, np.int64)
        colrow = np.empty(nn, np.int64)
        crow_of = np.empty(nn, np.int64)
        t_id = 0
        rows_in = 0
        nodes_in = 0
        for i in range(nn):
            r = r_n[i]
            if rows_in + r > 128:
                t_id += 1
                rows_in = 0
                nodes_in = 0
            tile_of[i] = t_id
            colrow[i] = rows_in
            crow_of[i] = nodes_in
            rows_in += r
            nodes_in += 1
        packs.append(dict(n0=n0, n1=n1, nn=nn, d=d, r_n=r_n, tile_of=tile_of,
                          colrow=colrow, crow_of=crow_of, nt=t_id + 1))

    NT = -(-max(p["nt"] for p in packs) // TC) * TC
    return packs, ssrc, cum, NT


def _expand_core(p, ssrc, cum, NT, keyT16, valT16, qT16):
    """Build the per-core expanded fp16 operand arrays."""
    n0 = p["n0"]
    nn = p["nn"]
    d, r_n = p["d"], p["r_n"]
    rows_total = NT * 128

    row_node = np.repeat(np.arange(nn), r_n)
    starts = np.concatenate([[0], np.cumsum(r_n)])[:-1]
    row_k = np.arange(len(row_node)) - np.repeat(starts, r_n)
    row_slot = (np.repeat(p["tile_of"], r_n) * 128
                + np.repeat(p["colrow"], r_n) + row_k)
    row_deg = np.clip(np.repeat(d, r_n) - row_k * D_PAD, 0, D_PAD)
    row_e0 = cum[n0 + row_node] + row_k * D_PAD
    j = np.arange(D_PAD)[None, :]
    valid = j < row_deg[:, None]
    eidx = np.minimum(row_e0[:, None] + j, E - 1)
    srcv = ssrc[eidx]

    # kvx[0:64, col]=key.T[src], [64:128]=value.T[src]; col=T*1024+s*128+p
    kvx = np.zeros((128, NT * 1024), np.float16)
    T_of = row_slot // 128
    p_of = row_slot % 128
    cols = T_of[:, None] * 1024 + j * 128 + p_of[:, None]
    cv = cols[valid]
    sv = srcv[valid]
    kvx[0:64, cv] = keyT16[:, sv]
    kvx[64:128, cv] = valT16[:, sv]

    # qx [65, NT*128], ones row for bias folding
    qx = np.zeros((65, NT * 128), np.float16)
    qx[64, :] = 1.0
    qx[0:64, row_slot] = qT16[:, n0 + row_node]

    # additive mask [128, NT*32], col = T*32 + s*4 + h
    mrow = np.full((rows_total, D_PAD), MASKV, np.float16)
    mrow[row_slot] = np.where(valid, np.float16(0.0), np.float16(MASKV))
    m4 = np.repeat(mrow.reshape(NT, 128, D_PAD)[:, :, :, None], H, axis=3)
    msk = np.ascontiguousarray(
        m4.transpose(1, 0, 2, 3).reshape(128, NT * D_PAD * H))

    # per-row node-column one-hot [128, NT*128] (host-built, DMA'd in)
    crow_slot = np.zeros(rows_total, np.int32)
    crow_slot[row_slot] = np.repeat(p["crow_of"], r_n).astype(np.int32)
    crw = crow_slot.reshape(NT, 128).T  # [p, T]
    ohx = np.ascontiguousarray(
        (crw[:, :, None] == np.arange(128)[None, None, :])
        .astype(np.float16).reshape(128, NT * 128))

    return dict(kvx=kvx, qx=qx, msk=msk, ohx=ohx)


def _build_program(NT):
    import concourse.bass as bass
    import concourse.tile as tile
    from concourse import bacc, mybir

    f32 = mybir.dt.float32
    f16 = mybir.dt.float16
    i32 = mybir.dt.int32
    AO = mybir.AluOpType

    nc = bacc.Bacc("TRN2", target_bir_lowering=False, debug=False,
                   num_devices=NCORES)

    kvx = nc.dram_tensor("kvx", [128, NT * 1024], f16, kind="ExternalInput").ap()
    qx = nc.dram_tensor("qx", [65, NT * 128], f16, kind="ExternalInput").ap()
    mskd = nc.dram_tensor("msk", [128, NT * D_PAD * H], f16, kind="ExternalInput").ap()
    ohxd = nc.dram_tensor("ohx", [128, NT * 128], f16, kind="ExternalInput").ap()
    wkvd = nc.dram_tensor("wkv", [128, 128], f16, kind="ExternalInput").ap()
    wqd = nc.dram_tensor("wq", [65, 68], f16, kind="ExternalInput").ap()
    wod = nc.dram_tensor("wo", [DIM, DIM], f16, kind="ExternalInput").ap()
    bod = nc.dram_tensor("bo", [128, DIM], f32, kind="ExternalInput").ap()
    comb = nc.dram_tensor("comb", [NT * 128, DIM], f32, kind="ExternalOutput").ap()

    ST = NT // TC

    def apx(t, dims, extra_off=0):
        a = t[:]
        return bass.AP(a.tensor, a.offset + extra_off, [list(a.ap[0])] + dims)

    with tile.TileContext(nc) as tc, ExitStack() as ctx, \
            nc.allow_low_precision("fp16 edge softmax within 2e-2 tolerance"):
        consts = ctx.enter_context(tc.tile_pool(name="consts", bufs=1))
        ld = ctx.enter_context(tc.tile_pool(name="ld", bufs=4))
        work = ctx.enter_context(tc.tile_pool(name="work", bufs=4))
        adexp = ctx.enter_context(tc.tile_pool(name="adexp", bufs=2 * TC + 2))
        bwork = ctx.enter_context(tc.tile_pool(name="bwork", bufs=6))
        pstp = ctx.enter_context(tc.tile_pool(name="pstp", bufs=2, space="PSUM"))
        qpsp = ctx.enter_context(tc.tile_pool(name="qpsp", bufs=2, space="PSUM"))
        cpsp = ctx.enter_context(tc.tile_pool(name="cpsp", bufs=2, space="PSUM"))

        from concourse.masks import make_identity

        wkv_sb = consts.tile([128, 128], f16)
        nc.sync.dma_start(wkv_sb[:], wkvd[:, :])
        wq_sb = consts.tile([65, 68], f16)
        nc.sync.dma_start(wq_sb[:], wqd[:, :])
        wo_sb = consts.tile([DIM, DIM], f16)
        nc.sync.dma_start(wo_sb[:], wod[:, :])
        bo_sb = consts.tile([128, DIM], f32)
        nc.sync.dma_start(bo_sb[:], bod[:, :])
        ident = consts.tile([128, 128], f16)
        make_identity(nc, ident[:])
        esh = consts.tile([128, 1], f32)
        nc.vector.memset(esh[:], ESHIFT)

        # state carried from supertile st-1 for deferred combine phase
        prev = None

        def emit_A(st):
            kvld = ld.tile([128, TC * 1024], f16, tag="kvld")
            nc.sync.dma_start(kvld[:], kvx[:, st * TC * 1024:(st + 1) * TC * 1024])
            qld = ld.tile([65, TC * 128], f16, tag="qld")
            nc.scalar.dma_start(qld[:], qx[:, st * TC * 128:(st + 1) * TC * 128])
            mld = ld.tile([128, TC * D_PAD * H], f16, tag="mld")
            nc.scalar.dma_start(
                mld[:], mskd[:, st * TC * D_PAD * H:(st + 1) * TC * D_PAD * H])
            ohld = ld.tile([128, TC * 128], f16, tag="ohld")
            nc.scalar.dma_start(ohld[:], ohxd[:, st * TC * 128:(st + 1) * TC * 128])

            adex_l = []
            def tile_A(t):
                # q~ projection: [128 rows, 68] (64 feats + 4 bk-fold cols)
                qp = qpsp.tile([128, 68], f32, space="PSUM", tag="qps")
                nc.tensor.matmul(out=qp[:], lhsT=qld[:, t * 128:(t + 1) * 128],
                                 rhs=wq_sb[:], start=True, stop=True)
                q16 = work.tile([128, 68], f16, tag="q16")
                nc.scalar.copy(q16[:], qp[:])

                # per-edge [k^ || v] projection: 8 slots -> PSUM [128,8,128]
                pst = pstp.tile([128, D_PAD, 128], f32, space="PSUM", tag="pst")
                for sl in range(D_PAD):
                    nc.tensor.matmul(
                        out=pst[:, sl, :],
                        lhsT=kvld[:, (t * D_PAD + sl) * 128:
                                  (t * D_PAD + sl + 1) * 128],
                        rhs=wkv_sb[:], start=True, stop=True)

                # scores: prod = k^ * q~ (slot-bcast), reduce per head
                prod = work.tile([128, D_PAD, DIM], f16, tag="prod")
                nc.vector.tensor_tensor(
                    out=prod[:],
                    in0=apx(pst, [[128, D_PAD], [1, DIM]]),
                    in1=apx(q16, [[0, D_PAD], [1, DIM]]),
                    op=AO.mult)
                sco = work.tile([128, D_PAD, H], f16, tag="sco")
                nc.vector.tensor_reduce(
                    out=sco[:],
                    in_=apx(prod, [[DIM, D_PAD], [DK, H], [1, DK]]),
                    axis=mybir.AxisListType.X, op=AO.add)
                # + mask, + per-head q.bk correction
                nc.vector.tensor_tensor(
                    out=sco[:], in0=sco[:],
                    in1=apx(mld, [[H, D_PAD], [1, H]], extra_off=t * D_PAD * H),
                    op=AO.add)
                nc.gpsimd.tensor_tensor(
                    out=sco[:], in0=sco[:],
                    in1=apx(q16, [[0, D_PAD], [1, H]], extra_off=DIM),
                    op=AO.add)

                # exp (scale 1/sqrt(dk), constant shift); adex = [exp*v || exp]
                adex = adexp.tile([128, D_PAD, DIM + H], f16, tag="adex")
                exe = work.tile([128, D_PAD, DIM], f16, tag="exe")
                nc.scalar.activation(
                    out=exe[:],
                    in_=apx(sco, [[H, D_PAD], [1, H], [0, DK]]),
                    func=mybir.ActivationFunctionType.Exp,
                    scale=1.0 / np.sqrt(DK), bias=esh[:])
                nc.scalar.activation(
                    out=apx(adex, [[DIM + H, D_PAD], [1, H]], extra_off=DIM),
                    in_=sco[:],
                    func=mybir.ActivationFunctionType.Exp,
                    scale=1.0 / np.sqrt(DK), bias=esh[:])
                vt16 = work.tile([128, D_PAD, DIM], f16, tag="vt16")
                nc.scalar.copy(vt16[:], apx(pst, [[128, D_PAD], [1, DIM]],
                                            extra_off=DIM))
                nc.gpsimd.tensor_tensor(
                    out=apx(adex, [[DIM + H, D_PAD], [1, DIM]]),
                    in0=vt16[:], in1=exe[:], op=AO.mult)
                adex_l.append(adex)
            return dict(st=st, adex=adex_l, ohld=ohld, tile_A=tile_A)

        def emit_B_tile(state, t):
            st = state["st"]
            ohld = state["ohld"]
            if True:
                adex = state["adex"][t]
                oh = ohld[:, t * 128:(t + 1) * 128]
                # one PSUM bank shared by the whole B chain:
                # f32 elems [0:136] = slot-pair combine, f16 elems [272:400]
                # = transpose out, f32 elems [200:264] = Wo matmul out
                mega = cpsp.tile([128, 512], f32, space="PSUM", tag="bmega")
                cp = apx(mega, [[DIM + H, 2], [1, DIM + H]])
                meg16 = mega.bitcast(f16)
                # combine rows -> node columns: 4 chained matmuls over slot
                # pairs, pairwise sums land in [128, 2, 68]
                for s4 in range(4):
                    nc.tensor.matmul(
                        out=cp, lhsT=oh,
                        rhs=adex[:, 2 * s4:2 * s4 + 2, :],
                        start=(s4 == 0), stop=(s4 == 3))
                cpf = bwork.tile([128, DIM + H], f32, tag="cpf")
                nc.vector.tensor_reduce(
                    out=cpf[:],
                    in_=apx(mega, [[1, DIM + H], [DIM + H, 2]]),
                    axis=mybir.AxisListType.X, op=AO.add)
                rd = bwork.tile([128, H], f32, tag="rd")
                nc.vector.reciprocal(rd[:], cpf[:, DIM:DIM + H])
                nrm = bwork.tile([128, DIM], f16, tag="nrm")
                nc.gpsimd.tensor_tensor(
                    out=nrm[:], in0=cpf[:, 0:DIM],
                    in1=apx(rd, [[1, H], [0, DK]]),
                    op=AO.mult)
                # transpose + output projection (Wo mixes heads, so it must
                # run after the per-head normalization)
                tps = meg16[0:DIM, 272:400]
                nc.tensor.transpose(out=tps, in_=nrm[:], identity=ident[:])
                nrmT = bwork.tile([DIM, 128], f16, tag="nrmT")
                nc.scalar.copy(nrmT[:], tps)
                ops_ = mega[:, 200:264]
                nc.tensor.matmul(out=ops_, lhsT=nrmT[:], rhs=wo_sb[:],
                                 start=True, stop=True)
                osb = bwork.tile([128, DIM], f32, tag="osb")
                nc.vector.tensor_tensor(
                    out=osb[:], in0=ops_, in1=bo_sb[:], op=AO.add)
                T = st * TC + t
                nc.sync.dma_start(comb[T * 128:(T + 1) * 128, :], osb[:])

        for st in range(ST):
            state = emit_A(st)
            for t in range(TC):
                state["tile_A"](t)
                if prev is not None:
                    emit_B_tile(prev, t)
            prev = state
        for t in range(TC):
            emit_B_tile(prev, t)

    nc.compile()
    return nc


def kernel(**inputs):
    from concourse.bass_utils import run_bass_kernel_spmd

    query = np.asarray(inputs["query"], np.float32)
    key = np.asarray(inputs["key"], np.float32)
    value = np.asarray(inputs["value"], np.float32)
    src = np.asarray(inputs["src"])
    dst = np.asarray(inputs["dst"])
    Wq = np.asarray(inputs["Wq"], np.float32)
    bq = np.asarray(inputs["bq"], np.float32)
    Wk = np.asarray(inputs["Wk"], np.float32)
    bk = np.asarray(inputs["bk"], np.float32)
    Wv = np.asarray(inputs["Wv"], np.float32)
    bv = np.asarray(inputs["bv"], np.float32)
    Wo = np.asarray(inputs["Wo"], np.float32)
    bo = np.asarray(inputs["bo"], np.float32)

    packs, ssrc, cum, NT = _host_prep(src, dst)
    nc = _build_program(NT)

    keyT16 = np.ascontiguousarray(key.T).astype(np.float16)
    valT16 = np.ascontiguousarray(value.T).astype(np.float16)
    qT16 = np.ascontiguousarray(query.T).astype(np.float16)

    # weight packing with bias folding (v stays in head space; Wo is applied
    # on-device after the per-head normalization)
    wkv = np.zeros((128, 128), np.float16)
    wkv[0:64, 0:64] = Wk.T
    wkv[64:128, 64:128] = Wv.T
    wq = np.zeros((65, 68), np.float16)
    wq[0:64, 0:64] = Wq.T
    wq[64, 0:64] = bq
    for h in range(H):
        sl = slice(h * DK, (h + 1) * DK)
        wq[0:64, 64 + h] = Wq[sl, :].T @ bk[sl]
        wq[64, 64 + h] = bq[sl] @ bk[sl]
    bo_eff = (bo + Wo @ bv).astype(np.float32)
    bo_b = np.broadcast_to(bo_eff, (128, DIM)).astype(np.float32).copy()

    in_maps = []
    for p in packs:
        ex = _expand_core(p, ssrc, cum, NT, keyT16, valT16, qT16)
        in_maps.append(dict(kvx=ex["kvx"], qx=ex["qx"], msk=ex["msk"],
                            ohx=ex["ohx"], wkv=wkv, wq=wq,
                            wo=Wo.T.astype(np.float16), bo=bo_b))

    trace = bool(int(os.environ.get("KERNEL_TRACE", "0")))
    res = run_bass_kernel_spmd(
        nc, in_maps, core_ids=list(range(NCORES)), trace=trace,
        tmpdir=os.environ.get("KERNEL_TRACE_DIR") or None,
    )
    kernel.last_results = res

    out = np.empty((N, DIM), np.float32)
    for p, r in zip(packs, res.results):
        rows = p["tile_of"] * 128 + p["crow_of"]
        out[p["n0"]:p["n1"]] = r["comb"][rows]
        z = p["d"] == 0
        if z.any():
            out[p["n0"]:p["n1"]][z] = bo
    return out
